# revision 33
# baseline (speedup 1.0000x reference)
"""Trainium2 Bass kernel for ConcentrationLoss (fp8 e4m3 streaming version).

Math (per batch element b, fully independent across b):
    g      = grid[b] viewed as (2, 4096)            # channels x pixels
    coord1 = g @ aff[b]                             # (2, 4096), the heavy op
    view coord1 as (2, 64, 64); extract 8x8 windows stride 4 -> 15x15 windows
    loss contribution = sum over windows w of [ sum_{p in w} x_p^2 - (sum_{p in w} x_p)^2 / 64 ]
    final = sum_b contribution_b / (8 * 2 * 225 * 64)

Sharding: batch b -> core b (8 cores). The kernel is HBM-bandwidth bound on
streaming aff, so aff is quantized host-side to fp8 e4m3 (16MB/core instead of
64MB): the loss is a variance over 230K window elements, so the quantization
noise averages to a ~4e-3 relative shift, far inside the 2e-2 gate.

Device pipeline per core:
  - aff is host-packed into [slab, p, kpair, ko, n] fp8 tile order; each
    512-column slab is streamed as two contiguous 1MB DMAs (kpair 0-7, 8-15).
  - Matmuls run in DoubleRow perf mode (fp8-only, 2 K-rows/cycle): each
    instruction consumes a (128, 2, 512) aff AP against a (128, 2, 2) grid
    weight AP (grid also e4m3; its quantization adds ~2e-3 to the loss).
  - Per finished PSUM bank (2, 512): ACT squares it, then overlapping-AP
    tensor_reduces produce the w-direction window sums of x and x^2;
    h-direction window rows follow incrementally, and each new chunk of full
    window sums is immediately squared and reduced into a per-window-row
    accumulator so the end-of-stream serial chain is short.
  - The final (2, 2) output holds sum(SSq) and sum(S^2) per channel.
    Host: loss_b = sum_c [ sumSSq_c - sumS2_c / 64 ], summed over cores.
"""

import numpy as np

B = 8
C = 2
H = W = 64
PIX = H * W  # 4096, contraction dim
WIN = 8
STRIDE = 4
OH = OW = 15
KC = PIX // 128   # 32 contraction chunks of 128
KP = KC // 2      # 16 DoubleRow chunk-pairs
NSLAB = PIX // 512  # 8 column slabs == psum banks
ROWS_PER_BANK = 512 // W  # 8 image rows per slab
AFF_BUFS = 15     # one SBUF buffer per 1MB transfer: every dma_start issues
                  # up front, so the stream is never issue-limited

_CACHE = {}


def _split_multi_waits(nc, limit=1):
    """The walrus build in this toolchain rejects instructions carrying more
    than one sync wait (any template: CTRL, S3_LW, ...). Tile's scheduler
    freely emits multi-wait instructions. Post-process the scheduled BIR:
    hoist excess waits onto one-wait NoOps inserted immediately before the
    instruction on the same engine (sequencer waits are conjunctive and
    blocking, so semantics are identical)."""
    import concourse.mybir as mybir

    n_split = 0
    for f in nc.m.functions:
        for b in f.blocks:
            insts = b.instructions  # live view
            i = 0
            while i < len(insts):
                inst = insts[i]
                si = inst.sync_info
                if si is not None and len(si.on_wait) > limit:
                    waits = list(si.on_wait)
                    extra, keep = waits[:-limit], waits[-limit:]
                    for w in extra:
                        nop = mybir.InstNoOp(name=f"SWS-{n_split}")
                        n_split += 1
                        nop.engine = inst.engine
                        nop.sync_info = mybir.SyncInfo(on_wait=[w], on_update=[])
                        insts.insert(i, nop)
                        i += 1
                    inst.sync_info = mybir.SyncInfo(
                        on_wait=keep, on_update=si.on_update
                    )
                i += 1
    return n_split


def _build_nc():
    import concourse.bass as bass
    import concourse.mybir as mybir
    import concourse.tile as tile

    f32 = mybir.dt.float32
    f8 = mybir.dt.float8e4
    nc = bass.Bass()
    # aff pre-packed on host: [slab, p, kpair, ko, n] so each half-slab DMA
    # (kpair 0-7 / 8-15) is one contiguous 1MB block with 8KB/partition runs
    aff = nc.dram_tensor("aff", [NSLAB, 128, KP, 2, 512], f8, kind="ExternalInput")
    # gt[p, ko, kpair, c]: g channel c at k = 256*kp + 128*ko + p
    gt = nc.dram_tensor("gt", [128, 2 * KP * C], f8, kind="ExternalInput")
    out = nc.dram_tensor("out", [C, 2, NSLAB], f32, kind="ExternalOutput")

    # DMA plan per slab: (kp0, kp count). Slab 7 tapers so the end-of-stream
    # dependency chain hangs off a small final transfer.
    PLAN = [[(0, 8), (8, 8)]] * (NSLAB - 1) + [[(0, 8), (8, 4), (12, 2), (14, 2)]]

    with tile.TileContext(nc) as tc:
        with (
            tc.tile_pool(name="small", bufs=1) as small,
            tc.tile_pool(name="affp", bufs=1) as affp,
            tc.tile_pool(name="ps1", bufs=1, space="PSUM") as ps1,
        ):
            # consts go through SWDGE (gpsimd) so they never queue behind the
            # big aff stream on the HWDGE ring
            gt_sb = small.tile([128, 2, KP, C], f8)
            nc.gpsimd.dma_start(out=gt_sb, in_=gt[:, :])

            NBLK = W // STRIDE  # 16 non-overlapping 4-col blocks per row

            # stage-1 block sums for the whole image, x and x^2 paths
            p4y_sb = small.tile([C, H, NBLK], f32)
            p4q_sb = small.tile([C, H, NBLK], f32)
            s_sb = small.tile([C, OH * OW], f32)    # full window sums
            acc_sb = small.tile([C, 2, NSLAB], f32)  # per-slab partials
            s2scr = small.tile([C, 2 * OW], f32)    # stt scratch

            def blocked(ap):
                """[part, row, blk, 4] non-overlapping AP over (C, 8*64):
                stage-1 of the window sums (reduce 4-col blocks)."""
                return bass.AP(
                    tensor=ap.tensor,
                    offset=ap.offset,
                    ap=[list(ap.ap[0]), [W, ROWS_PER_BANK], [STRIDE, NBLK],
                        [1, STRIDE]],
                )

            # Window sums from block sums: window (i, j) = sum over dh of 8
            # rows 4i..4i+7 and 2 blocks j, j+1 of p4[row, blk].
            # After bank n, rows up to 8n+7 exist -> window rows {2n-1, 2n}
            # (plus row 0 for n=0) become computable.
            sv = s_sb.rearrange("c (i j) -> c i j", j=OW)

            def win5(src, i0, cnt):
                """5D AP [part, i, j, dh, blk] over a p4 buffer."""
                ap = src[:, :, :]
                return bass.AP(
                    tensor=ap.tensor,
                    offset=ap.offset + i0 * STRIDE * NBLK,
                    ap=[list(ap.ap[0]), [STRIDE * NBLK, cnt], [1, OW],
                        [NBLK, WIN], [1, 2]],
                )

            # issue EVERY aff transfer up front, alternating the two HWDGE
            # rings (sync + scalar): the stream is never issue-limited and
            # both descriptor generators feed the 16 SDMA engines. Safe for
            # the scalar ring because all dma_starts precede any square in
            # its queue.
            tiles = []
            ti = 0
            for s in range(NSLAB):
                for kp0, kcnt in PLAN[s]:
                    nbufs = {8: AFF_BUFS, 4: 1, 2: 2}[kcnt]
                    at = affp.tile(
                        [128, kcnt, 2, 512], f8, tag=f"aff{kcnt}", bufs=nbufs
                    )
                    ring = nc.sync if ti % 2 == 0 else nc.scalar
                    ring.dma_start(out=at, in_=aff[s, :, kp0:kp0 + kcnt])
                    tiles.append(at)
                    ti += 1

            ti = 0
            for s in range(NSLAB):
                # one full PSUM bank per slab: a [128, 512] f32 tile is
                # exactly one 2KB-per-partition bank, so slab s's post reads
                # never share a single-port bank with slab s+1's matmuls
                c1bf = ps1.tile([128, 512], f32, tag="bank", bufs=NSLAB, name=f"c1b{s}")
                c1b = c1bf[0:C, :]
                for kp0, kcnt in PLAN[s]:
                    at = tiles[ti]
                    ti += 1
                    for kp in range(kp0, kp0 + kcnt):
                        nc.tensor.matmul(
                            c1b,
                            lhsT=gt_sb[:, :, kp, :],
                            rhs=at[:, kp - kp0, :, :],
                            start=(kp == 0),
                            stop=(kp == KP - 1),
                            perf_mode=mybir.MatmulPerfMode.DoubleRow,
                        )

                # post-process bank s (overlaps the remaining aff stream).
                # DVE stage-1 takes the single-port bank first, then ACT's
                # square; all window math runs off the p4 block-sum buffers.
                p4yd = p4y_sb[:, s * ROWS_PER_BANK:(s + 1) * ROWS_PER_BANK, :]
                p4qd = p4q_sb[:, s * ROWS_PER_BANK:(s + 1) * ROWS_PER_BANK, :]
                sq = small.tile([C, 512], f32, tag="sq", bufs=2)
                nc.vector.reduce_sum(
                    out=p4yd, in_=blocked(c1b), axis=mybir.AxisListType.X,
                )
                nc.scalar.square(out=sq, in_=c1b)
                if s == 0:
                    i0, cnt = 0, 1
                else:
                    i0, cnt = 2 * s - 1, 2
                # window sums S for the fresh window rows (kept per-window
                # for squaring), then sum(S^2) via one stt with accumulate
                nc.vector.reduce_sum(
                    out=sv[:, i0:i0 + cnt, :], in_=win5(p4y_sb, i0, cnt),
                    axis=mybir.AxisListType.XY,
                )
                nc.vector.scalar_tensor_tensor(
                    out=s2scr[:, 0:cnt * OW],
                    in0=s_sb[:, i0 * OW:(i0 + cnt) * OW], scalar=1.0,
                    in1=s_sb[:, i0 * OW:(i0 + cnt) * OW],
                    op0=mybir.AluOpType.mult, op1=mybir.AluOpType.mult,
                    accum_out=acc_sb[:, 1, s:s + 1],
                )
                nc.vector.reduce_sum(
                    out=p4qd, in_=blocked(sq[:, :]), axis=mybir.AxisListType.X,
                )
                # sum of all SSq in this row chunk in a single 5D reduce
                nc.vector.reduce_sum(
                    out=acc_sb[:, 0, s:s + 1], in_=win5(p4q_sb, i0, cnt),
                    axis=mybir.AxisListType.XYZW,
                )

            # host sums the 8 per-slab partials; skipping the on-device final
            # reduce removes one sem-latency link from the end-of-stream chain
            nc.sync.dma_start(out=out[:, :, :], in_=acc_sb)
    _split_multi_waits(nc)
    return nc


def _pack_aff(aff_b):
    """aff_b (4096, 4096) f32 -> e4m3 packed [slab, p, kpair, ko, n]."""
    import ml_dtypes

    a8 = aff_b.astype(ml_dtypes.float8_e4m3)
    # [kp, ko, p, s, n] -> [s, p, kp, ko, n]
    a8 = a8.reshape(KP, 2, 128, NSLAB, 512).transpose(3, 2, 0, 1, 4)
    return np.ascontiguousarray(a8)


def _gt_host(grid_b):
    """grid_b (64, 64, 2) -> gt[p, ko, kp, c] fp8.
    g[c, k] = grid_b.reshape(4096, 2)[k, c]; k = 256*kp + 128*ko + p."""
    import ml_dtypes

    g = np.ascontiguousarray(grid_b, dtype=np.float32).reshape(PIX, C)  # [k, c]
    gt = g.astype(ml_dtypes.float8_e4m3)
    gt = gt.reshape(KP, 2, 128, C).transpose(2, 1, 0, 3)  # [p, ko, kp, c]
    return np.ascontiguousarray(gt).reshape(128, 2 * KP * C)


def run_cores(aff, grid, trace=False):
    """Compile (cached) and run the per-core bass kernel on cores 0..7.

    Returns the BassKernelResults from run_bass_kernel_spmd."""
    from concourse.bass_utils import run_bass_kernel_spmd

    if "nc" not in _CACHE:
        _CACHE["nc"] = _build_nc()
    nc = _CACHE["nc"]

    in_maps = []
    for b in range(B):
        in_maps.append(
            {
                "aff": _pack_aff(np.ascontiguousarray(aff[b], dtype=np.float32)),
                "gt": _gt_host(grid[b]),
            }
        )
    return run_bass_kernel_spmd(nc, in_maps, core_ids=list(range(B)), trace=trace)


def kernel(aff, grid):
    aff = np.asarray(aff, dtype=np.float32)
    grid = np.asarray(grid, dtype=np.float32)
    res = run_cores(aff, grid)
    total = 0.0
    for b in range(B):
        o = res.results[b]["out"].astype(np.float64)
        total += o[:, 0, :].sum() - o[:, 1, :].sum() / (WIN * WIN)
    total /= B * C * OH * OW * WIN * WIN
    return np.asarray(total, dtype=np.float32)


# revision 34
# speedup vs baseline: 1.1148x; 1.1148x over previous
"""Trainium2 Bass kernel for ConcentrationLoss (fp8 e4m3 streaming version).

Math (per batch element b, fully independent across b):
    g      = grid[b] viewed as (2, 4096)            # channels x pixels
    coord1 = g @ aff[b]                             # (2, 4096), the heavy op
    view coord1 as (2, 64, 64); extract 8x8 windows stride 4 -> 15x15 windows
    loss contribution = sum over windows w of [ sum_{p in w} x_p^2 - (sum_{p in w} x_p)^2 / 64 ]
    final = sum_b contribution_b / (8 * 2 * 225 * 64)

Sharding: batch b -> core b (8 cores). The kernel is HBM-bandwidth bound on
streaming aff, so aff is quantized host-side to fp8 e4m3 (16MB/core instead of
64MB): the loss is a variance over 230K window elements, so the quantization
noise averages to a ~4e-3 relative shift, far inside the 2e-2 gate.

Device pipeline per core:
  - aff is host-packed into [slab, p, kpair, ko, n] fp8 tile order; each
    512-column slab is streamed as two contiguous 1MB DMAs (kpair 0-7, 8-15).
  - Matmuls run in DoubleRow perf mode (fp8-only, 2 K-rows/cycle): each
    instruction consumes a (128, 2, 512) aff AP against a (128, 2, 2) grid
    weight AP (grid also e4m3; its quantization adds ~2e-3 to the loss).
  - Per finished PSUM bank (2, 512): ACT squares it, then overlapping-AP
    tensor_reduces produce the w-direction window sums of x and x^2;
    h-direction window rows follow incrementally, and each new chunk of full
    window sums is immediately squared and reduced into a per-window-row
    accumulator so the end-of-stream serial chain is short.
  - The final (2, 2) output holds sum(SSq) and sum(S^2) per channel.
    Host: loss_b = sum_c [ sumSSq_c - sumS2_c / 64 ], summed over cores.
"""

import numpy as np

B = 8
C = 2
H = W = 64
PIX = H * W  # 4096, contraction dim
WIN = 8
STRIDE = 4
OH = OW = 15
KC = PIX // 128   # 32 contraction chunks of 128
KP = KC // 2      # 16 DoubleRow chunk-pairs
NSLAB = PIX // 512  # 8 column slabs == psum banks
ROWS_PER_BANK = 512 // W  # 8 image rows per slab
AFF_BUFS = 15     # one SBUF buffer per 1MB transfer: every dma_start issues
                  # up front, so the stream is never issue-limited

_CACHE = {}


def _split_multi_waits(nc, limit=1):
    """The walrus build in this toolchain rejects instructions carrying more
    than one sync wait (any template: CTRL, S3_LW, ...). Tile's scheduler
    freely emits multi-wait instructions. Post-process the scheduled BIR:
    hoist excess waits onto one-wait NoOps inserted immediately before the
    instruction on the same engine (sequencer waits are conjunctive and
    blocking, so semantics are identical)."""
    import concourse.mybir as mybir

    n_split = 0
    for f in nc.m.functions:
        for b in f.blocks:
            insts = b.instructions  # live view
            i = 0
            while i < len(insts):
                inst = insts[i]
                si = inst.sync_info
                if si is not None and len(si.on_wait) > limit:
                    waits = list(si.on_wait)
                    extra, keep = waits[:-limit], waits[-limit:]
                    for w in extra:
                        nop = mybir.InstNoOp(name=f"SWS-{n_split}")
                        n_split += 1
                        nop.engine = inst.engine
                        nop.sync_info = mybir.SyncInfo(on_wait=[w], on_update=[])
                        insts.insert(i, nop)
                        i += 1
                    inst.sync_info = mybir.SyncInfo(
                        on_wait=keep, on_update=si.on_update
                    )
                i += 1
    return n_split


def _build_nc():
    import concourse.bass as bass
    import concourse.mybir as mybir
    import concourse.tile as tile

    f32 = mybir.dt.float32
    f8 = mybir.dt.float8e4
    nc = bass.Bass()
    # aff pre-packed on host: [slab, p, kpair, ko, n] so each half-slab DMA
    # (kpair 0-7 / 8-15) is one contiguous 1MB block with 8KB/partition runs
    aff = nc.dram_tensor("aff", [NSLAB, 128, KP, 2, 512], f8, kind="ExternalInput")
    # gt[p, ko, kpair, c]: g channel c at k = 256*kp + 128*ko + p
    gt = nc.dram_tensor("gt", [128, 2 * KP * C], f8, kind="ExternalInput")
    out = nc.dram_tensor("out", [C, 2, NSLAB], f32, kind="ExternalOutput")

    # DMA plan per slab: (kp0, kp count). Slab 7 tapers so the end-of-stream
    # dependency chain hangs off a small final transfer.
    PLAN = ([[(0, 16)]] * (NSLAB - 2)
            + [[(0, 8), (8, 8)]]
            + [[(0, 8), (8, 4), (12, 2), (14, 2)]])

    with tile.TileContext(nc) as tc:
        with (
            tc.tile_pool(name="small", bufs=1) as small,
            tc.tile_pool(name="affp", bufs=1) as affp,
            tc.tile_pool(name="ps1", bufs=1, space="PSUM") as ps1,
        ):
            # consts go through SWDGE (gpsimd) so they never queue behind the
            # big aff stream on the HWDGE ring
            gt_sb = small.tile([128, 2, KP, C], f8)
            nc.gpsimd.dma_start(out=gt_sb, in_=gt[:, :])

            NBLK = W // STRIDE  # 16 non-overlapping 4-col blocks per row

            # stage-1 block sums for the whole image, x and x^2 paths
            p4y_sb = small.tile([C, H, NBLK], f32)
            p4q_sb = small.tile([C, H, NBLK], f32)
            s_sb = small.tile([C, OH * OW], f32)    # full window sums
            acc_sb = small.tile([C, 2, NSLAB], f32)  # per-slab partials
            s2scr = small.tile([C, 2 * OW], f32)    # stt scratch

            def blocked(ap):
                """[part, row, blk, 4] non-overlapping AP over (C, 8*64):
                stage-1 of the window sums (reduce 4-col blocks)."""
                return bass.AP(
                    tensor=ap.tensor,
                    offset=ap.offset,
                    ap=[list(ap.ap[0]), [W, ROWS_PER_BANK], [STRIDE, NBLK],
                        [1, STRIDE]],
                )

            # Window sums from block sums: window (i, j) = sum over dh of 8
            # rows 4i..4i+7 and 2 blocks j, j+1 of p4[row, blk].
            # After bank n, rows up to 8n+7 exist -> window rows {2n-1, 2n}
            # (plus row 0 for n=0) become computable.
            sv = s_sb.rearrange("c (i j) -> c i j", j=OW)

            def win5(src, i0, cnt):
                """5D AP [part, i, j, dh, blk] over a p4 buffer."""
                ap = src[:, :, :]
                return bass.AP(
                    tensor=ap.tensor,
                    offset=ap.offset + i0 * STRIDE * NBLK,
                    ap=[list(ap.ap[0]), [STRIDE * NBLK, cnt], [1, OW],
                        [NBLK, WIN], [1, 2]],
                )

            for s in range(NSLAB):
                # one full PSUM bank per slab: a [128, 512] f32 tile is
                # exactly one 2KB-per-partition bank, so slab s's post reads
                # never share a single-port bank with slab s+1's matmuls
                c1bf = ps1.tile([128, 512], f32, tag="bank", bufs=NSLAB, name=f"c1b{s}")
                c1b = c1bf[0:C, :]
                for kp0, kcnt in PLAN[s]:
                    nbufs = {16: 6, 8: 3, 4: 1, 2: 2}[kcnt]
                    at = affp.tile(
                        [128, kcnt, 2, 512], f8, tag=f"aff{kcnt}", bufs=nbufs
                    )
                    nc.sync.dma_start(out=at, in_=aff[s, :, kp0:kp0 + kcnt])
                    for kp in range(kp0, kp0 + kcnt):
                        nc.tensor.matmul(
                            c1b,
                            lhsT=gt_sb[:, :, kp, :],
                            rhs=at[:, kp - kp0, :, :],
                            start=(kp == 0),
                            stop=(kp == KP - 1),
                            perf_mode=mybir.MatmulPerfMode.DoubleRow,
                        )

                # post-process bank s (overlaps the remaining aff stream).
                # DVE stage-1 takes the single-port bank first, then ACT's
                # square; all window math runs off the p4 block-sum buffers.
                p4yd = p4y_sb[:, s * ROWS_PER_BANK:(s + 1) * ROWS_PER_BANK, :]
                p4qd = p4q_sb[:, s * ROWS_PER_BANK:(s + 1) * ROWS_PER_BANK, :]
                sq = small.tile([C, 512], f32, tag="sq", bufs=2)
                nc.vector.reduce_sum(
                    out=p4yd, in_=blocked(c1b), axis=mybir.AxisListType.X,
                )
                nc.scalar.square(out=sq, in_=c1b)
                if s == 0:
                    i0, cnt = 0, 1
                else:
                    i0, cnt = 2 * s - 1, 2
                # window sums S for the fresh window rows (kept per-window
                # for squaring), then sum(S^2) via one stt with accumulate
                nc.vector.reduce_sum(
                    out=sv[:, i0:i0 + cnt, :], in_=win5(p4y_sb, i0, cnt),
                    axis=mybir.AxisListType.XY,
                )
                nc.vector.scalar_tensor_tensor(
                    out=s2scr[:, 0:cnt * OW],
                    in0=s_sb[:, i0 * OW:(i0 + cnt) * OW], scalar=1.0,
                    in1=s_sb[:, i0 * OW:(i0 + cnt) * OW],
                    op0=mybir.AluOpType.mult, op1=mybir.AluOpType.mult,
                    accum_out=acc_sb[:, 1, s:s + 1],
                )
                nc.vector.reduce_sum(
                    out=p4qd, in_=blocked(sq[:, :]), axis=mybir.AxisListType.X,
                )
                # sum of all SSq in this row chunk in a single 5D reduce
                nc.vector.reduce_sum(
                    out=acc_sb[:, 0, s:s + 1], in_=win5(p4q_sb, i0, cnt),
                    axis=mybir.AxisListType.XYZW,
                )

            # host sums the 8 per-slab partials; skipping the on-device final
            # reduce removes one sem-latency link from the end-of-stream chain
            nc.sync.dma_start(out=out[:, :, :], in_=acc_sb)
    _split_multi_waits(nc)
    return nc


def _pack_aff(aff_b):
    """aff_b (4096, 4096) f32 -> e4m3 packed [slab, p, kpair, ko, n]."""
    import ml_dtypes

    a8 = aff_b.astype(ml_dtypes.float8_e4m3)
    # [kp, ko, p, s, n] -> [s, p, kp, ko, n]
    a8 = a8.reshape(KP, 2, 128, NSLAB, 512).transpose(3, 2, 0, 1, 4)
    return np.ascontiguousarray(a8)


def _gt_host(grid_b):
    """grid_b (64, 64, 2) -> gt[p, ko, kp, c] fp8.
    g[c, k] = grid_b.reshape(4096, 2)[k, c]; k = 256*kp + 128*ko + p."""
    import ml_dtypes

    g = np.ascontiguousarray(grid_b, dtype=np.float32).reshape(PIX, C)  # [k, c]
    gt = g.astype(ml_dtypes.float8_e4m3)
    gt = gt.reshape(KP, 2, 128, C).transpose(2, 1, 0, 3)  # [p, ko, kp, c]
    return np.ascontiguousarray(gt).reshape(128, 2 * KP * C)


def run_cores(aff, grid, trace=False):
    """Compile (cached) and run the per-core bass kernel on cores 0..7.

    Returns the BassKernelResults from run_bass_kernel_spmd."""
    from concourse.bass_utils import run_bass_kernel_spmd

    if "nc" not in _CACHE:
        _CACHE["nc"] = _build_nc()
    nc = _CACHE["nc"]

    in_maps = []
    for b in range(B):
        in_maps.append(
            {
                "aff": _pack_aff(np.ascontiguousarray(aff[b], dtype=np.float32)),
                "gt": _gt_host(grid[b]),
            }
        )
    return run_bass_kernel_spmd(nc, in_maps, core_ids=list(range(B)), trace=trace)


def kernel(aff, grid):
    aff = np.asarray(aff, dtype=np.float32)
    grid = np.asarray(grid, dtype=np.float32)
    res = run_cores(aff, grid)
    total = 0.0
    for b in range(B):
        o = res.results[b]["out"].astype(np.float64)
        total += o[:, 0, :].sum() - o[:, 1, :].sum() / (WIN * WIN)
    total /= B * C * OH * OW * WIN * WIN
    return np.asarray(total, dtype=np.float32)


# revision 36
# speedup vs baseline: 1.2932x; 1.1601x over previous
"""Trainium2 Bass kernel for ConcentrationLoss (fp8 e4m3 streaming version).

Math (per batch element b, fully independent across b):
    g      = grid[b] viewed as (2, 4096)            # channels x pixels
    coord1 = g @ aff[b]                             # (2, 4096), the heavy op
    view coord1 as (2, 64, 64); extract 8x8 windows stride 4 -> 15x15 windows
    loss contribution = sum over windows w of [ sum_{p in w} x_p^2 - (sum_{p in w} x_p)^2 / 64 ]
    final = sum_b contribution_b / (8 * 2 * 225 * 64)

Sharding: batch b -> core b (8 cores). The kernel is HBM-bandwidth bound on
streaming aff, so aff is quantized host-side to fp8 e4m3 (16MB/core instead of
64MB): the loss is a variance over 230K window elements, so the quantization
noise averages to a ~4e-3 relative shift, far inside the 2e-2 gate.

Device pipeline per core:
  - aff is host-packed into [slab, p, kpair, ko, n] fp8 tile order; each
    512-column slab is streamed as two contiguous 1MB DMAs (kpair 0-7, 8-15).
  - Matmuls run in DoubleRow perf mode (fp8-only, 2 K-rows/cycle): each
    instruction consumes a (128, 2, 512) aff AP against a (128, 2, 2) grid
    weight AP (grid also e4m3; its quantization adds ~2e-3 to the loss).
  - Per finished PSUM bank (2, 512): ACT squares it, then overlapping-AP
    tensor_reduces produce the w-direction window sums of x and x^2;
    h-direction window rows follow incrementally, and each new chunk of full
    window sums is immediately squared and reduced into a per-window-row
    accumulator so the end-of-stream serial chain is short.
  - The final (2, 2) output holds sum(SSq) and sum(S^2) per channel.
    Host: loss_b = sum_c [ sumSSq_c - sumS2_c / 64 ], summed over cores.
"""

import numpy as np

B = 8
C = 2
H = W = 64
PIX = H * W  # 4096, contraction dim
WIN = 8
STRIDE = 4
OH = OW = 15
KC = PIX // 128   # 32 contraction chunks of 128
KP = KC // 2      # 16 DoubleRow chunk-pairs
NSLAB = PIX // 512  # 8 column slabs == psum banks
ROWS_PER_BANK = 512 // W  # 8 image rows per slab
AFF_BUFS = 15     # one SBUF buffer per 1MB transfer: every dma_start issues
                  # up front, so the stream is never issue-limited

_CACHE = {}


def _split_multi_waits(nc, limit=1):
    """The walrus build in this toolchain rejects instructions carrying more
    than one sync wait (any template: CTRL, S3_LW, ...). Tile's scheduler
    freely emits multi-wait instructions. Post-process the scheduled BIR:
    hoist excess waits onto one-wait NoOps inserted immediately before the
    instruction on the same engine (sequencer waits are conjunctive and
    blocking, so semantics are identical)."""
    import concourse.mybir as mybir

    n_split = 0
    for f in nc.m.functions:
        for b in f.blocks:
            insts = b.instructions  # live view
            i = 0
            while i < len(insts):
                inst = insts[i]
                si = inst.sync_info
                if si is not None and len(si.on_wait) > limit:
                    waits = list(si.on_wait)
                    extra, keep = waits[:-limit], waits[-limit:]
                    for w in extra:
                        nop = mybir.InstNoOp(name=f"SWS-{n_split}")
                        n_split += 1
                        nop.engine = inst.engine
                        nop.sync_info = mybir.SyncInfo(on_wait=[w], on_update=[])
                        insts.insert(i, nop)
                        i += 1
                    inst.sync_info = mybir.SyncInfo(
                        on_wait=keep, on_update=si.on_update
                    )
                i += 1
    return n_split


def _build_nc():
    import concourse.bass as bass
    import concourse.mybir as mybir
    import concourse.tile as tile

    f32 = mybir.dt.float32
    f8 = mybir.dt.float8e4
    nc = bass.Bass()
    # aff pre-packed on host: [slab, p, kpair, ko, n] so each half-slab DMA
    # (kpair 0-7 / 8-15) is one contiguous 1MB block with 8KB/partition runs
    aff = nc.dram_tensor("aff", [NSLAB, 128, KP, 2, 512], f8, kind="ExternalInput")
    # gt[p, ko, kpair, c]: g channel c at k = 256*kp + 128*ko + p
    gt = nc.dram_tensor("gt", [128, 2 * KP * C], f8, kind="ExternalInput")
    out = nc.dram_tensor("out", [C, 2, NSLAB], f32, kind="ExternalOutput")

    # DMA plan per slab: (kp0, kp count). Slab 7 tapers so the end-of-stream
    # dependency chain hangs off a small final transfer.
    PLAN = [[(0, 8), (8, 8)]] * (NSLAB - 1) + [[(0, 8), (8, 4), (12, 2), (14, 2)]]

    with tile.TileContext(nc) as tc:
        with (
            tc.tile_pool(name="small", bufs=1) as small,
            tc.tile_pool(name="affp", bufs=1) as affp,
            tc.tile_pool(name="ps1", bufs=1, space="PSUM") as ps1,
        ):
            # consts go through SWDGE (gpsimd) so they never queue behind the
            # big aff stream on the HWDGE ring
            gt_sb = small.tile([128, 2, KP, C], f8)
            nc.gpsimd.dma_start(out=gt_sb, in_=gt[:, :])

            NBLK = W // STRIDE  # 16 non-overlapping 4-col blocks per row

            # stage-1 block sums for the whole image, x and x^2 paths
            p4y_sb = small.tile([C, H, NBLK], f32)
            p4q_sb = small.tile([C, H, NBLK], f32)
            s_sb = small.tile([C, OH * OW], f32)    # full window sums
            acc_sb = small.tile([C, 2, NSLAB], f32)  # per-slab partials
            s2scr = small.tile([C, 2 * OW], f32)    # stt scratch

            def blocked(ap):
                """[part, row, blk, 4] non-overlapping AP over (C, 8*64):
                stage-1 of the window sums (reduce 4-col blocks)."""
                return bass.AP(
                    tensor=ap.tensor,
                    offset=ap.offset,
                    ap=[list(ap.ap[0]), [W, ROWS_PER_BANK], [STRIDE, NBLK],
                        [1, STRIDE]],
                )

            # Window sums from block sums: window (i, j) = sum over dh of 8
            # rows 4i..4i+7 and 2 blocks j, j+1 of p4[row, blk].
            # After bank n, rows up to 8n+7 exist -> window rows {2n-1, 2n}
            # (plus row 0 for n=0) become computable.
            sv = s_sb.rearrange("c (i j) -> c i j", j=OW)

            def win5(src, i0, cnt):
                """5D AP [part, i, j, dh, blk] over a p4 buffer."""
                ap = src[:, :, :]
                return bass.AP(
                    tensor=ap.tensor,
                    offset=ap.offset + i0 * STRIDE * NBLK,
                    ap=[list(ap.ap[0]), [STRIDE * NBLK, cnt], [1, OW],
                        [NBLK, WIN], [1, 2]],
                )

            for s in range(NSLAB):
                # one full PSUM bank per slab: a [128, 512] f32 tile is
                # exactly one 2KB-per-partition bank, so slab s's post reads
                # never share a single-port bank with slab s+1's matmuls
                c1bf = ps1.tile([128, 512], f32, tag="bank", bufs=NSLAB, name=f"c1b{s}")
                c1b = c1bf[0:C, :]
                for kp0, kcnt in PLAN[s]:
                    nbufs = {8: AFF_BUFS, 4: 1, 2: 2}[kcnt]
                    at = affp.tile(
                        [128, kcnt, 2, 512], f8, tag=f"aff{kcnt}", bufs=nbufs
                    )
                    nc.sync.dma_start(out=at, in_=aff[s, :, kp0:kp0 + kcnt])
                    for kp in range(kp0, kp0 + kcnt):
                        nc.tensor.matmul(
                            c1b,
                            lhsT=gt_sb[:, :, kp, :],
                            rhs=at[:, kp - kp0, :, :],
                            start=(kp == 0),
                            stop=(kp == KP - 1),
                            perf_mode=mybir.MatmulPerfMode.DoubleRow,
                        )

                # post-process bank s (overlaps the remaining aff stream).
                # DVE stage-1 takes the single-port bank first, then ACT's
                # square; all window math runs off the p4 block-sum buffers.
                p4yd = p4y_sb[:, s * ROWS_PER_BANK:(s + 1) * ROWS_PER_BANK, :]
                p4qd = p4q_sb[:, s * ROWS_PER_BANK:(s + 1) * ROWS_PER_BANK, :]
                sq = small.tile([C, 512], f32, tag="sq", bufs=2)
                nc.vector.reduce_sum(
                    out=p4yd, in_=blocked(c1b), axis=mybir.AxisListType.X,
                )
                nc.scalar.square(out=sq, in_=c1b)
                if s == 0:
                    i0, cnt = 0, 1
                else:
                    i0, cnt = 2 * s - 1, 2
                # window sums S for the fresh window rows (kept per-window
                # for squaring), then sum(S^2) via one stt with accumulate
                nc.vector.reduce_sum(
                    out=sv[:, i0:i0 + cnt, :], in_=win5(p4y_sb, i0, cnt),
                    axis=mybir.AxisListType.XY,
                )
                nc.vector.scalar_tensor_tensor(
                    out=s2scr[:, 0:cnt * OW],
                    in0=s_sb[:, i0 * OW:(i0 + cnt) * OW], scalar=1.0,
                    in1=s_sb[:, i0 * OW:(i0 + cnt) * OW],
                    op0=mybir.AluOpType.mult, op1=mybir.AluOpType.mult,
                    accum_out=acc_sb[:, 1, s:s + 1],
                )
                nc.vector.reduce_sum(
                    out=p4qd, in_=blocked(sq[:, :]), axis=mybir.AxisListType.X,
                )
                # sum of all SSq in this row chunk in a single 5D reduce
                nc.vector.reduce_sum(
                    out=acc_sb[:, 0, s:s + 1], in_=win5(p4q_sb, i0, cnt),
                    axis=mybir.AxisListType.XYZW,
                )

            # host sums the 8 per-slab partials; skipping the on-device final
            # reduce removes one sem-latency link from the end-of-stream chain
            nc.sync.dma_start(out=out[:, :, :], in_=acc_sb)
    _split_multi_waits(nc)
    return nc


def _pack_aff(aff_b):
    """aff_b (4096, 4096) f32 -> e4m3 packed [slab, p, kpair, ko, n]."""
    import ml_dtypes

    a8 = aff_b.astype(ml_dtypes.float8_e4m3)
    # [kp, ko, p, s, n] -> [s, p, kp, ko, n]
    a8 = a8.reshape(KP, 2, 128, NSLAB, 512).transpose(3, 2, 0, 1, 4)
    return np.ascontiguousarray(a8)


def _gt_host(grid_b):
    """grid_b (64, 64, 2) -> gt[p, ko, kp, c] fp8.
    g[c, k] = grid_b.reshape(4096, 2)[k, c]; k = 256*kp + 128*ko + p."""
    import ml_dtypes

    g = np.ascontiguousarray(grid_b, dtype=np.float32).reshape(PIX, C)  # [k, c]
    gt = g.astype(ml_dtypes.float8_e4m3)
    gt = gt.reshape(KP, 2, 128, C).transpose(2, 1, 0, 3)  # [p, ko, kp, c]
    return np.ascontiguousarray(gt).reshape(128, 2 * KP * C)


def run_cores(aff, grid, trace=False):
    """Compile (cached) and run the per-core bass kernel on cores 0..7.

    Returns the BassKernelResults from run_bass_kernel_spmd."""
    from concourse.bass_utils import run_bass_kernel_spmd

    if "nc" not in _CACHE:
        _CACHE["nc"] = _build_nc()
    nc = _CACHE["nc"]

    in_maps = []
    for b in range(B):
        in_maps.append(
            {
                "aff": _pack_aff(np.ascontiguousarray(aff[b], dtype=np.float32)),
                "gt": _gt_host(grid[b]),
            }
        )
    return run_bass_kernel_spmd(nc, in_maps, core_ids=list(range(B)), trace=trace)


def kernel(aff, grid):
    aff = np.asarray(aff, dtype=np.float32)
    grid = np.asarray(grid, dtype=np.float32)
    res = run_cores(aff, grid)
    total = 0.0
    for b in range(B):
        o = res.results[b]["out"].astype(np.float64)
        total += o[:, 0, :].sum() - o[:, 1, :].sum() / (WIN * WIN)
    total /= B * C * OH * OW * WIN * WIN
    return np.asarray(total, dtype=np.float32)
